# revision 9
# baseline (speedup 1.0000x reference)
"""Trainium2 Bass kernel for nn_Downsample_Spa: sigma-conv + gaussian unfold downsample.

Math (per batch image, one NeuronCore each; batch of 8 -> 8 cores):
  xp = reflect_pad(x)                                  # [64,130,130]
  sigma[o,p] = clamp(BN(conv3x3(xp))[o,p], 1e-4)       # at stride-2 positions p only
  graw[o,p]  = exp(-0.5*d2[o]/sigma^2 - ln64) / sigma  # /64 guard; cancels in the ratio
  gn[o,p]    = graw[o,p] / sum_o graw[o,p]             # normalized BEFORE broadcast
  out[c,p]   = sum_o gn[o,p]*xp[c,p+off(o)]

Design (v2):
 - partitions = (row-half hh, channel c) = 128; host pre-pads (reflect), fp16, and
   parity-splits columns into 2 planes (w=2j / w=2j+1); tap (i,2) = plane0 at j+1.
   2.2MB upload vs 3.3 for the 3-plane layout.
 - two groups of 16 out-rows per half (2048 positions, N=1024 matmuls): conv = 9
   accumulating fp16 MMs (block-diag weights, M=18 computes both halves); g pipeline
   [18,1024] fp32: clamp, fast-reciprocal, ACT Square+Exp, gb=et*inv (bf16);
   S via one M=18 block-diag-ones MM (replicates the o-sum to all 9 rows); gn =
   gb * recip(S) (bf16).  Broadcast: 9 one-hot MMs [18->128] per group, N=1024.
 - PE instruction stream is gap-free (warmup MMs; conv g1 issued behind conv g0;
   broadcasts behind convs) so HAM stays un-throttled (2.4GHz) for the whole kernel.
 - unfold: products y_o = x_tap * grep_o split across DVE (PSUM-direct), GpSimd, and
   ACT-evac+DVE-2x; fp16 pair tree; center tap fp32 end-to-end; out fp32 per group.
"""

import os
import sys

import numpy as np

if "/opt/trn_rl_repo" not in sys.path:
    sys.path.insert(0, "/opt/trn_rl_repo")

K = 3
BN_EPS = 1e-5
SIGMA_MIN = 1e-4
GSCALE_LN = float(np.log(64.0))   # graw scaled by 1/64 (folded into exp bias)
N, C, H, W = 8, 64, 128, 128
HO = WO = 64
HH = 2
RS = 65                  # padded-row slots per partition-half
HOC = 32                 # out rows per half
NGRP = 2
GR = HOC // NGRP         # 16 out rows (per half) per group
NPOS = GR * WO           # 1024 matmul columns per group
PL = 2                   # x col-parity planes: w=2j / w=2j+1
JW = 66                  # j slots per plane (65 used max, 66 for alignment)

# f32 consts tensor columns
_D2 = 0                  # -0.5*d2[o] per (hh,o)
_BC = 1                  # bn_bias - sigma_min
_LB = 2                  # exp bias: constant -ln(64) per partition
_NCC = 3

_STATE = {}


def _build_consts(conv_w, bn_gamma, bn_beta, bn_mean, bn_var):
    s = (bn_gamma / np.sqrt(bn_var + BN_EPS)).astype(np.float32)
    wf = conv_w.astype(np.float32) * s[:, None, None, None]           # [9,64,3,3]
    bias = (bn_beta - bn_mean * s).astype(np.float32)

    cst = np.zeros((18, _NCC), np.float32)
    d2 = np.array([(kk // 3 - 1) ** 2 + (kk % 3 - 1) ** 2 for kk in range(9)], np.float32)
    for hh in range(HH):
        cst[hh * 9:hh * 9 + 9, _D2] = -0.5 * d2
        cst[hh * 9:hh * 9 + 9, _BC] = bias - SIGMA_MIN
        cst[hh * 9:hh * 9 + 9, _LB] = -GSCALE_LN

    # conv weights, block-diagonal per tap: win[k=hh*64+c, tap*18 + hh*9+o]
    win = np.zeros((128, 9 * 18), np.float16)
    for tap in range(9):
        i, j = tap // 3, tap % 3
        for hh in range(HH):
            win[hh * 64:hh * 64 + 64, tap * 18 + hh * 9:tap * 18 + hh * 9 + 9] = \
                wf[:, :, i, j].T.astype(np.float16)

    import ml_dtypes
    # one-hot broadcast weights: gin[k=hh*9+o, tap*128 + hh*64+c]
    gin = np.zeros((18, 9 * 128), ml_dtypes.bfloat16)
    for hh in range(HH):
        for tap in range(9):
            gin[hh * 9 + tap, tap * 128 + hh * 64:tap * 128 + hh * 64 + 64] = 1.0
    # block-diag ones for the o-sum replicate: wS[k=(hh,o), m=(hh,o')] = [hh==hh']
    wS = np.zeros((18, 18), ml_dtypes.bfloat16)
    for hh in range(HH):
        wS[hh * 9:hh * 9 + 9, hh * 9:hh * 9 + 9] = 1.0
    return cst, win, gin, wS


def _build_bass(for_sim=False):
    import concourse.bass as bass
    import concourse.tile as tile
    from concourse import mybir

    f32 = mybir.dt.float32
    f16 = mybir.dt.float16
    bf16 = mybir.dt.bfloat16
    MULT = mybir.AluOpType.mult
    ADD = mybir.AluOpType.add
    MAX = mybir.AluOpType.max
    AF = mybir.ActivationFunctionType

    if for_sim:
        nc = bass.Bass("TRN2", target_bir_lowering=False, detect_race_conditions=False)
    else:
        from concourse import bacc
        nc = bacc.Bacc()
    xin = nc.dram_tensor("xin", [128, RS, PL, JW], f16, kind="ExternalInput")
    cin = nc.dram_tensor("cin", [18, _NCC], f32, kind="ExternalInput")
    win = nc.dram_tensor("win", [128, 9 * 18], f16, kind="ExternalInput")
    gin = nc.dram_tensor("gin", [18, 9 * 128], bf16, kind="ExternalInput")
    sin = nc.dram_tensor("sin", [18, 18], bf16, kind="ExternalInput")
    out = nc.dram_tensor("out", [128, HOC, WO], f32, kind="ExternalOutput")

    with tile.TileContext(nc) as tc:
        from contextlib import ExitStack
        with ExitStack() as ctx:
            big = ctx.enter_context(tc.tile_pool(name="big", bufs=1))
            gsb = ctx.enter_context(tc.tile_pool(name="gsb", bufs=2))
            y_p = ctx.enter_context(tc.tile_pool(name="y", bufs=2))
            # PSUM: sig pool [18,1024]=2 banks x2; grep pool [128,1024]=2 banks x2
            ps_s = ctx.enter_context(tc.tile_pool(name="ps_s", bufs=2, space="PSUM"))
            ps_g = ctx.enter_context(tc.tile_pool(name="ps_g", bufs=2, space="PSUM"))

            ws = big.tile([128, 9 * 18], f16)
            nc.sync.dma_start(out=ws[:], in_=win[:])
            cs = big.tile([18, _NCC], f32)
            nc.gpsimd.dma_start(out=cs[:], in_=cin[:])
            gs = big.tile([18, 9 * 128], bf16)
            nc.gpsimd.dma_start(out=gs[:], in_=gin[:])
            ss = big.tile([18, 18], bf16)
            nc.gpsimd.dma_start(out=ss[:], in_=sin[:])

            # whole padded image in one tile; 4 row-chunk DMAs arrive progressively
            xs = big.tile([128, RS, PL, JW], f16)
            RCH = ((0, 17), (17, 33), (33, 49), (49, 65))
            for r0, r1 in RCH:
                nc.sync.dma_start(out=xs[:, r0:r1], in_=xin[:, r0:r1])

            def xtap(tap, g):
                # [128, GR, 64] fp16 step-1 view for tap (i,b) in group g
                i, b = tap // 3, tap % 3
                pl, jo = (0, 1) if b == 2 else (b, 0)
                r0 = i + 2 * GR * g
                return xs[:, r0:r0 + 2 * GR - 1:2, pl, jo:jo + WO]

            # ---- PE warm-up on the early-arriving weights tile (HAM ramp) ----
            wu = ps_s.tile([18, NPOS], f32, tag="sig")
            for _ in range(8):
                nc.tensor.matmul(wu[:, 0:162], ws[:, 0:18], ws[:, 0:162],
                                 start=True, stop=True)

            # ---- conv: per-group sigma [18,1024] in PSUM ----
            sig_t = []
            for g in range(NGRP):
                sig = ps_s.tile([18, NPOS], f32, tag="sig")
                for h in range(2):          # PSUM bank halves (N=512 per MM)
                    for tap in range(9):
                        nc.tensor.matmul(
                            sig[:, 512 * h:512 * (h + 1)],
                            ws[:, tap * 18:(tap + 1) * 18],
                            xtap(tap, g)[:, 8 * h:8 * (h + 1), :],
                            start=(tap == 0), stop=(tap == 8),
                        )
                sig_t.append(sig)

            def g_emit(sig):
                # clamp + gaussian pipeline -> gb (bf16), all [18, 1024]
                sc = gsb.tile([18, NPOS], f32, tag="sc")
                nc.vector.tensor_scalar(out=sc[:], in0=sig[:],
                                        scalar1=cs[:, _BC:_BC + 1],
                                        scalar2=float(SIGMA_MIN),
                                        op0=ADD, op1=MAX)
                inv = gsb.tile([18, NPOS], f32, tag="inv")
                nc.vector.reciprocal_approx_fast(out=inv[:], in_=sc[:])
                qt = gsb.tile([18, NPOS], f32, tag="qt")
                nc.scalar.activation(out=qt[:], in_=inv[:], func=AF.Square)
                et = gsb.tile([18, NPOS], f32, tag="et")
                nc.scalar.activation(out=et[:], in_=qt[:], func=AF.Exp,
                                     scale=cs[:, _D2:_D2 + 1],
                                     bias=cs[:, _LB:_LB + 1])
                gb = gsb.tile([18, NPOS], bf16, tag="gb")
                nc.vector.tensor_tensor(out=gb[:], in0=et[:], in1=inv[:], op=MULT)
                return gb

            def gn_emit(gb):
                # S replicate via block-diag ones MM, then gn = gb * (1/S)
                srep = ps_s.tile([18, NPOS], f32, tag="sig")
                for h in range(2):
                    nc.tensor.matmul(srep[:, 512 * h:512 * (h + 1)], ss[:],
                                     gb[:, 512 * h:512 * (h + 1)],
                                     start=True, stop=True)
                rrs = gsb.tile([18, NPOS], f32, tag="rrs")
                nc.vector.reciprocal_approx_fast(out=rrs[:], in_=srep[:])
                gn = gsb.tile([18, NPOS], bf16, tag="gn")
                nc.vector.tensor_tensor(out=gn[:], in0=gb[:], in1=rrs[:], op=MULT)
                return gn

            def unfold_emit(g, gn, late):
                # taps order in yt slots: [0,1,2,3,5,6,7,8]; center (4) fp32 separate
                yt = y_p.tile([128, 8, GR, WO], f16, tag="yt")
                y4 = y_p.tile([128, GR, WO], f32, tag="y4")
                gc = y_p.tile([128, 4, GR, WO], f16, tag="gc")

                def bcast(tap):
                    grep = ps_g.tile([128, GR, WO], f32, tag="grep")
                    for h in range(2):
                        nc.tensor.matmul(grep[:, 8 * h:8 * (h + 1), :],
                                         gs[:, tap * 128:(tap + 1) * 128],
                                         gn[:, 512 * h:512 * (h + 1)],
                                         start=True, stop=True)
                    return grep

                slot = {0: 0, 1: 1, 2: 2, 3: 3, 5: 4, 6: 5, 7: 6, 8: 7}
                # DVE PSUM-direct taps (GpSimd cannot read PSUM on TRN2)
                for tap in (0, 2, 6, 8):
                    grep = bcast(tap)
                    nc.vector.tensor_tensor(out=yt[:, slot[tap]], in0=xtap(tap, g),
                                            in1=grep[:], op=MULT)
                # ACT-evac taps 1,3,5,7 -> fp16; mults on GpSimd (1,7) / DVE 2x (3,5)
                for k, tap in enumerate((1, 3, 5, 7)):
                    grep = bcast(tap)
                    nc.scalar.activation(out=gc[:, k], in_=grep[:], func=AF.Copy)
                g4 = bcast(4)
                nc.gpsimd.tensor_tensor(out=yt[:, 1], in0=xtap(1, g), in1=gc[:, 0], op=MULT)
                nc.vector.tensor_tensor(out=yt[:, 3], in0=xtap(3, g), in1=gc[:, 1], op=MULT)
                nc.vector.tensor_tensor(out=yt[:, 4], in0=xtap(5, g), in1=gc[:, 2], op=MULT)
                nc.gpsimd.tensor_tensor(out=yt[:, 6], in0=xtap(7, g), in1=gc[:, 3], op=MULT)
                # center tap fp32
                nc.vector.tensor_tensor(out=y4[:], in0=xtap(4, g), in1=g4[:], op=MULT)

                # pair tree (fp16) + center
                t4 = y_p.tile([128, 4, GR, WO], f16, tag="t4")
                nc.vector.tensor_tensor(out=t4[:], in0=yt[:, 0:8:2], in1=yt[:, 1:8:2], op=ADD)
                t2 = y_p.tile([128, 2, GR, WO], f16, tag="t2")
                nc.vector.tensor_tensor(out=t2[:], in0=t4[:, 0:4:2], in1=t4[:, 1:4:2], op=ADD)
                t1 = y_p.tile([128, GR, WO], f16, tag="t1")
                (nc.vector if late else nc.gpsimd).tensor_tensor(
                    out=t1[:], in0=t2[:, 0], in1=t2[:, 1], op=ADD)
                acc = y_p.tile([128, GR, WO], f32, tag="acc")
                (nc.vector if late else nc.gpsimd).tensor_tensor(
                    out=acc[:], in0=t1[:], in1=y4[:], op=ADD)
                nc.sync.dma_start(out=out[:, GR * g:GR * (g + 1), :], in_=acc[:])

            gb0 = g_emit(sig_t[0])
            gb1 = g_emit(sig_t[1])
            gn0 = gn_emit(gb0)
            unfold_emit(0, gn0, late=False)
            gn1 = gn_emit(gb1)
            unfold_emit(1, gn1, late=True)

    if not for_sim and not nc.is_finalized():
        nc.finalize()
    return nc


def _prep_inputs(x, conv_w, bn_gamma, bn_beta, bn_mean, bn_var):
    cst, win, gin, wS = _build_consts(conv_w, bn_gamma, bn_beta, bn_mean, bn_var)
    xp = np.pad(np.asarray(x, np.float32), ((0, 0), (0, 0), (1, 1), (1, 1)),
                mode="reflect").astype(np.float16)                    # [8,64,130,130]
    in_maps = []
    for n in range(N):
        xc = np.concatenate([xp[n, :, 0:RS, :], xp[n, :, 64:64 + RS, :]], axis=0)
        xpl = np.zeros((128, RS, PL, JW), np.float16)
        xpl[:, :, 0, 0:65] = xc[:, :, 0:130:2]
        xpl[:, :, 1, 0:64] = xc[:, :, 1:129:2]
        in_maps.append({"xin": xpl, "cin": cst, "win": win, "gin": gin, "sin": wS})
    return in_maps


def _gather(results):
    out = np.empty((N, C, HO, WO), np.float32)
    for n in range(N):
        d = results[n]["out"]
        out[n, :, 0:HOC, :] = d[0:64]
        out[n, :, HOC:, :] = d[64:128]
    return out


def _enable_axon_trace():
    """Register the NTFF profile hook that this image's antenv lacks."""
    if _STATE.get("trace_hooked"):
        return
    import types
    import antenv
    from concourse import bass_utils
    mod = types.ModuleType("antenv.axon_hooks")
    mod._hook = None
    mod.set_axon_ntff_profile_hook = lambda h: setattr(mod, "_hook", h)
    mod.get_axon_ntff_profile_hook = lambda: mod._hook
    sys.modules["antenv.axon_hooks"] = mod
    antenv.axon_hooks = mod
    from trn_agent_boot.trn_boot import _ntff_profile_via_ctypes
    mod._hook = _ntff_profile_via_ctypes("/opt/axon/libaxon_pjrt.so")
    bass_utils.upload_artifacts = lambda tmpdir: tmpdir
    _STATE["trace_hooked"] = True


def run(x, conv_w, bn_gamma, bn_beta, bn_mean, bn_var, trace=False):
    from concourse.bass_utils import run_bass_kernel_spmd
    if trace:
        _enable_axon_trace()
    if "nc" not in _STATE:
        _STATE["nc"] = _build_bass()
    in_maps = _prep_inputs(x, conv_w, bn_gamma, bn_beta, bn_mean, bn_var)
    res = run_bass_kernel_spmd(_STATE["nc"], in_maps, list(range(N)), trace=trace)
    _STATE["last"] = res
    return _gather(res.results)


def kernel(x, conv_w, bn_gamma, bn_beta, bn_mean, bn_var):
    return run(x, conv_w, bn_gamma, bn_beta, bn_mean, bn_var,
               trace=bool(int(os.environ.get("KERNEL_TRACE", "0"))))
